# revision 17
# baseline (speedup 1.0000x reference)
"""Trainium2 Bass kernel for nn_AlignmentModule.

Data-parallel over batch: 8 samples -> 8 NeuronCores, one sample each.

Per-core computation (sample b):
  hh = conv1(k3, relu) -> conv2(k1) over h           (S=128, D=256)
  mm = conv1(k3, relu) -> conv2(k3, relu) -> conv3(k1) over m  (F=512, D=256)
  dist(s,f) = ||hh[s] - mm[f]||_2  via h2 + m2 - 2*hh.mm (matmul)
  log_softmax over s of (-dist) with row mask (no-max logsumexp: dist >= 0)
  out = beta_binomial_log_prior + log_softmax

Conv/dist/reduction matmuls run in bf16 (full-rate PE); the distance is
assembled in fp32 PSUM with h2 entering as the Ln activation's per-partition
bias and m2 / -lse broadcast via K=1 fp16 matmuls (small-magnitude rows).
sqrt(d2) = exp(0.5*ln(d2)) keeps every ScalarE transcendental in one
activation-table set (preloaded manually). Everything after feat-conv2 is
split into two F-halves (conv3 is 1x1 so no halo) so PE / ScalarE / VectorE
/ DMA stages of the two halves pipeline.

The beta-binomial prior factors as C + f1(k) + f2(j) + g(k+j) (all gammaln
arguments are integers); host builds a (128,512) fp32 tile
G = g(k+j) + C + f1(k) + f2(j) with math.lgamma. -inf regions
(k >= token_length, j >= feat_length) are data-independent given the length
scalars and are filled on host after the gather.
"""

import math
import os

import numpy as np

import concourse.bacc as bacc
import concourse.bass as bass
import concourse.mybir as mybir
from concourse import masks, tile
from concourse.bass_utils import run_bass_kernel_spmd

FP = mybir.dt.float32
BF = mybir.dt.bfloat16
HF = mybir.dt.float16
B, S, F, D, FEAT = 8, 128, 512, 256, 80
HH = F // 2
NEGB = -30000.0  # exp(NEGB + x) == 0 in fp32 for any realistic x
NLE_SET_ID = 6   # natural_log_exp_and_others in act_info.json

LAST_EXEC_NS = None
LAST_TRACE_PATH = None

_CACHED_NC = None


def _ensure_trace_hook():
    """Register the axon NTFF profile hook if the image's antenv lacks it."""
    import sys
    import types

    try:
        from antenv.axon_hooks import get_axon_ntff_profile_hook
        if get_axon_ntff_profile_hook() is not None:
            return True
    except ImportError:
        pass
    try:
        if "/root/.axon_site" not in sys.path:
            sys.path.insert(0, "/root/.axon_site")
        from trn_agent_boot.trn_boot import _ntff_profile_via_ctypes

        hook = _ntff_profile_via_ctypes("/opt/axon/libaxon_pjrt.so")
        mod = types.ModuleType("antenv.axon_hooks")
        holder = [hook]
        mod.get_axon_ntff_profile_hook = lambda: holder[0]
        mod.set_axon_ntff_profile_hook = lambda h: holder.__setitem__(0, h)
        sys.modules["antenv.axon_hooks"] = mod
        import concourse.bass_utils as bu

        bu.upload_artifacts = lambda tmpdir: "local://" + tmpdir
        return True
    except Exception:
        return False


def _build_nc():
    nc = bacc.Bacc("TRN2", debug=False, num_devices=B)

    def inp(name, shape, dt=FP):
        return nc.declare_dram_parameter(name, list(shape), dt, isOutput=False)

    # pack1: [mT (514) | fw1t (3*256)] on 80 partitions
    p1_d = inp("pack1", (FEAT, (F + 2) + 3 * D), BF)
    # pack2: [hT (2*130) | w1t c0 (768) | w1t c1 (768) | w2t (512)] on 128 partitions
    p2_d = inp("pack2", (128, 2 * (S + 2) + 2 * 3 * D + 2 * D), BF)
    # pack3: [fw2t c0 (768) | fw2t c1 (768) | fw3t (512)]
    p3_d = inp("pack3", (128, 2 * 3 * D + 2 * D), BF)
    bp_d = inp("biaspack", (128, 11))  # tb1x2 tb2x2 fb1x2 fb2x2 fb3x2 maskbias
    g_d = inp("gmat", (S, F))          # g(k+j) + C + f1(k) + f2(j)
    out_d = nc.declare_dram_parameter("out", [S, F], FP, isOutput=True)

    Act = mybir.ActivationFunctionType
    Alu = mybir.AluOpType

    def dap(d, offset, pairs):
        return bass.AP(d[:].tensor, offset, pairs)

    mm_ = nc.tensor.matmul

    with tile.TileContext(nc) as tc:
        with (
            tc.tile_pool(name="const", bufs=1) as cp,
            tc.tile_pool(name="wts", bufs=1) as wp,
            tc.tile_pool(name="act", bufs=1) as ap,
            tc.tile_pool(name="psA", bufs=2, space="PSUM") as psA,
            tc.tile_pool(name="psF", bufs=2, space="PSUM") as psF,
            tc.tile_pool(name="psR", bufs=1, space="PSUM") as psR,
            tc.tile_pool(name="psD", bufs=1, space="PSUM") as psD,
            tc.tile_pool(name="psB", bufs=1, space="PSUM") as psB,
        ):
            # preload the single activation-table set (ln+exp+relu+square+...)
            nc.scalar.add_instruction(
                mybir.InstLoadActFuncSet(
                    name=nc.get_next_instruction_name(),
                    act_func_set_id=NLE_SET_ID,
                    ins=[],
                    outs=[],
                )
            )

            # ---- input DMAs: three packed loads + biases + prior tile ----
            p1_sb = ap.tile([FEAT, (F + 2) + 3 * D], BF, tag="p1")
            nc.sync.dma_start(p1_sb[:], p1_d[:])
            p2_sb = ap.tile([128, 2 * (S + 2) + 2 * 3 * D + 2 * D], BF, tag="p2")
            nc.sync.dma_start(p2_sb[:], p2_d[:])
            bp_sb = cp.tile([128, 11], FP, tag="bp")
            nc.scalar.dma_start(bp_sb[:], bp_d[:])
            p3_sb = ap.tile([128, 2 * 3 * D + 2 * D], BF, tag="p3")
            nc.scalar.dma_start(p3_sb[:], p3_d[:])
            g_sb = ap.tile([S, F], FP, tag="g")
            nc.scalar.dma_start(g_sb[:], g_d[:])

            mT = p1_sb[:, 0:F + 2]
            def f1w(t, o):
                base = (F + 2) + t * D + o * 128
                return p1_sb[:, base:base + 128]
            hT = [p2_sb[:, c * (S + 2):(c + 1) * (S + 2)] for c in range(2)]
            def w1w(c, t, o):
                base = 2 * (S + 2) + c * 3 * D + t * D + o * 128
                return p2_sb[:, base:base + 128]
            def w2w(c, o):
                base = 2 * (S + 2) + 2 * 3 * D + c * D + o * 128
                return p2_sb[:, base:base + 128]
            def f2w(c, t, o):
                base = c * 3 * D + t * D + o * 128
                return p3_sb[:, base:base + 128]
            def f3w(c, o):
                base = 2 * 3 * D + c * D + o * 128
                return p3_sb[:, base:base + 128]

            tb1c = [bp_sb[:, i:i + 1] for i in (0, 1)]
            tb2c = [bp_sb[:, i:i + 1] for i in (2, 3)]
            fb1c = [bp_sb[:, i:i + 1] for i in (4, 5)]
            fb2c = [bp_sb[:, i:i + 1] for i in (6, 7)]
            fb3c = [bp_sb[:, i:i + 1] for i in (8, 9)]
            mb_col = bp_sb[:, 10:11]

            # ---- constants (warmup operands first) ----
            ones_r128 = cp.tile([1, 128], HF, tag="ones_r128")
            nc.gpsimd.memset(ones_r128[:], 1.0)
            warm_row = cp.tile([1, F], HF, tag="warm_row")
            nc.gpsimd.memset(warm_row[:], 1.0)
            ones_col = cp.tile([128, 1], BF, tag="ones_col")
            nc.gpsimd.memset(ones_col[:], 1.0)
            neg_r128 = cp.tile([1, 128], HF, tag="neg_r128")
            nc.gpsimd.memset(neg_r128[:], -1.0)

            # HAM warmup while the input DMAs land (PE would idle anyway)
            pb = psB.tile([128, F], FP, tag="b")
            for _ in range(6):
                mm_(pb[:], ones_r128[:], warm_row[:], start=True, stop=True)

            # ---- feat conv1 + conv2 (full width; k=3 halo) ----
            y1 = []
            for o in range(2):
                py = psF.tile([128, F], FP, tag="cf")
                for t in range(3):
                    mm_(py[:], f1w(t, o), mT[:, t:t + F],
                        start=(t == 0), stop=(t == 2))
                yp = ap.tile([128, F + 2], BF, tag=f"y1_{o}")
                nc.gpsimd.memset(yp[:, 0:1], 0.0)
                nc.gpsimd.memset(yp[:, F + 1:F + 2], 0.0)
                if o == 0:
                    nc.scalar.activation(yp[:, 1:F + 1], py[:], Act.Relu,
                                         bias=fb1c[o], scale=1.0)
                else:
                    nc.vector.tensor_scalar(
                        out=yp[:, 1:F + 1], in0=py[:], scalar1=fb1c[o], scalar2=0.0,
                        op0=Alu.add, op1=Alu.max,
                    )
                y1.append(yp)

            # ---- text conv stack (starts as soon as hT + w1 land) ----
            x1T = []
            for o in range(2):
                px = psA.tile([128, S], FP, tag="A")
                k = 0
                for t in range(3):
                    for c in range(2):
                        mm_(px[:], w1w(c, t, o),
                            hT[c][:, t:t + S], start=(k == 0), stop=(k == 5))
                        k += 1
                x = ap.tile([128, S], BF, tag=f"x1T_{o}")
                if o == 0:
                    nc.scalar.activation(x[:], px[:], Act.Relu, bias=tb1c[o], scale=1.0)
                else:
                    nc.vector.tensor_scalar(
                        out=x[:], in0=px[:], scalar1=tb1c[o], scalar2=0.0,
                        op0=Alu.add, op1=Alu.max,
                    )
                x1T.append(x)

            hhT = []
            hsq = []
            for o in range(2):
                px = psA.tile([128, S], FP, tag="A")
                for c in range(2):
                    mm_(px[:], w2w(c, o), x1T[c][:],
                        start=(c == 0), stop=(c == 1))
                hh = ap.tile([128, S], BF, tag=f"hhT_{o}")
                if o == 0:
                    nc.vector.tensor_scalar_add(hh[:], px[:], tb2c[o])
                else:
                    nc.scalar.activation(hh[:], px[:], Act.Identity, bias=tb2c[o], scale=1.0)
                hhT.append(hh)
                sq = ap.tile([128, S], BF, tag=f"hsq_{o}")
                if o == 0:
                    nc.scalar.activation(sq[:], hh[:], Act.Square)
                else:
                    nc.vector.tensor_mul(sq[:], hh[:], hh[:])
                hsq.append(sq)

            # h2 as a (128,1) column via N=1 matmuls (enters Ln as bias)
            ph2 = psA.tile([128, 1], FP, tag="A")
            for c in range(2):
                mm_(ph2[:], hsq[c][:], ones_col[:], start=(c == 0), stop=(c == 1))
            h2col = cp.tile([128, 1], FP, tag="h2col")
            nc.vector.tensor_copy(h2col[:], ph2[:])

            py2 = []
            for q in range(2):
                row = []
                for o in range(2):
                    py = psF.tile([128, HH], FP, tag="cf", name=f"c2_{q}_{o}")
                    k = 0
                    for t in range(3):
                        for c in range(2):
                            mm_(py[:], f2w(c, t, o),
                                y1[c][:, q * HH + t:q * HH + t + HH],
                                start=(k == 0), stop=(k == 5))
                            k += 1
                    row.append(py)
                py2.append(row)

            # ---- half-split pipeline: y2 relu -> conv3 -> dist -> softmax ----
            y2 = [ap.tile([128, F], BF, tag=f"y2_{o}", name=f"y2_{o}") for o in range(2)]
            mmT = [ap.tile([128, F], BF, tag=f"mmT_{o}", name=f"mmT_{o}") for o in range(2)]
            msq = [ap.tile([128, F], BF, tag=f"msq_{o}", name=f"msq_{o}") for o in range(2)]
            pm2 = psR.tile([1, F], FP, tag="rowm")
            m2n = cp.tile([1, F], HF, tag="m2n")
            pd = psD.tile([128, F], FP, tag="d")
            pcs = psR.tile([1, F], FP, tag="rowc")

            for q in range(2):
                sl = slice(q * HH, (q + 1) * HH)
                # y2 relu halves (conv3 is 1x1: no halo needed)
                for o in range(2):
                    if (o + q) % 2 == 0:
                        nc.vector.tensor_scalar(
                            out=y2[o][:, sl], in0=py2[q][o][:], scalar1=fb2c[o],
                            scalar2=0.0, op0=Alu.add, op1=Alu.max,
                        )
                    else:
                        nc.scalar.activation(y2[o][:, sl], py2[q][o][:], Act.Relu,
                                             bias=fb2c[o], scale=1.0)
                # conv3 halves
                for o in range(2):
                    pc3 = psA.tile([128, HH], FP, tag="A")
                    for c in range(2):
                        mm_(pc3[:], f3w(c, o), y2[c][:, sl],
                            start=(c == 0), stop=(c == 1))
                    if (o + q) % 2 == 0:
                        nc.scalar.activation(mmT[o][:, sl], pc3[:], Act.Identity,
                                             bias=fb3c[o], scale=1.0)
                        nc.vector.tensor_mul(msq[o][:, sl], mmT[o][:, sl], mmT[o][:, sl])
                    else:
                        nc.vector.tensor_scalar_add(mmT[o][:, sl], pc3[:], fb3c[o])
                        nc.scalar.activation(msq[o][:, sl], mmT[o][:, sl], Act.Square)

                # m2 half row, scaled by -0.5, fp16
                for c in range(2):
                    mm_(pm2[:, sl], ones_col[:], msq[c][:, sl],
                        start=(c == 0), stop=(c == 1))
                nc.vector.tensor_scalar_mul(m2n[:, sl], pm2[:, sl], -0.5)

                # dist psum = hh.mm - 0.5 m2 ; h2 enters via Ln bias
                mm_(pd[:, sl], hhT[0][:], mmT[0][:, sl], start=True, stop=False)
                mm_(pd[:, sl], hhT[1][:], mmT[1][:, sl], start=False, stop=False)
                mm_(pd[:, sl], ones_r128[:], m2n[:, sl], start=False, stop=True)

                tln = ap.tile([128, HH], FP, tag=f"tln_{q}")
                nc.scalar.activation(tln[:], pd[:, sl], Act.Ln, scale=-2.0,
                                     bias=h2col[:])
                dist = ap.tile([128, HH], FP, tag=f"dist_{q}")
                nc.scalar.activation(dist[:], tln[:], Act.Exp, scale=0.5)

                # E = exp(-dist + maskbias) (0 on masked rows)
                e_sb = ap.tile([128, HH], BF, tag=f"e_{q}")
                nc.scalar.activation(e_sb[:], dist[:], Act.Exp, bias=mb_col, scale=-1.0)

                # column logsumexp over partitions; -lse broadcast via -1 row
                mm_(pcs[:, sl], ones_col[:], e_sb[:], start=True, stop=True)
                lse = cp.tile([1, HH], HF, tag=f"lse_{q}")
                nc.scalar.activation(lse[:], pcs[:, sl], Act.Ln)
                mm_(pb[:, sl], neg_r128[:], lse[:], start=True, stop=True)

                # out = (G - dist) + pb
                fin1 = ap.tile([128, HH], FP, tag=f"fin1_{q}")
                nc.vector.scalar_tensor_tensor(
                    out=fin1[:], in0=dist[:], scalar=-1.0, in1=g_sb[:, sl],
                    op0=Alu.mult, op1=Alu.add,
                )
                out_sb = ap.tile([S, HH], FP, tag=f"outsb_{q}")
                nc.vector.tensor_add(out_sb[:], fin1[:], pb[:, sl])
                nc.sync.dma_start(out_d[:, sl], out_sb[:])

    nc.finalize()
    return nc


def _lgamma_table(n):
    # gl[i] = gammaln(i + 1) for i in 0..n-1  -> gammaln(k) = gl[k - 1]
    return np.array([math.lgamma(i + 1.0) for i in range(n)], dtype=np.float64)


_GL = _lgamma_table(2 * (S + F) + 8)
GLEN = S + F + 64  # g vector length >= S + F - 1


def _gln(x):
    # gammaln at integer x, clamped to >= 1 (gammaln(1) = 0)
    x = np.maximum(np.asarray(x, dtype=np.int64), 1)
    return _GL[x - 1]


def kernel(**inputs):
    global _CACHED_NC, LAST_EXEC_NS, LAST_TRACE_PATH
    if _CACHED_NC is None:
        _CACHED_NC = _build_nc()
    nc = _CACHED_NC

    BFNP = mybir.dt.np(BF)
    bf16 = lambda a: np.ascontiguousarray(np.asarray(a, np.float32).astype(BFNP))
    h = np.asarray(inputs["h"], np.float32)
    m = np.asarray(inputs["m"], np.float32)
    mask = np.asarray(inputs["mask"]).astype(bool)
    token_length = np.asarray(inputs["token_length"]).astype(np.int64)
    feat_length = np.asarray(inputs["feat_length"]).astype(np.int64)

    w1t = np.asarray(inputs["tw1"], np.float32).transpose(2, 1, 0)   # (3, D, D)
    w2t = np.asarray(inputs["tw2"], np.float32)[:, :, 0].T           # (D, D)
    fw1t = np.asarray(inputs["fw1"], np.float32).transpose(2, 1, 0)  # (3, 80, D)
    fw2t = np.asarray(inputs["fw2"], np.float32).transpose(2, 1, 0)  # (3, D, D)
    fw3t = np.asarray(inputs["fw3"], np.float32)[:, :, 0].T          # (D, D)
    # pack1 (80, 514+768): [mT | f1(t-major)] -- per-sample mT added below
    f1_flat = fw1t.transpose(1, 0, 2).reshape(FEAT, 3 * D)
    # pack2 (128, 260+1536+512): [hT | w1 c0 | w1 c1 | w2]
    w1_part = np.concatenate(
        [w1t[:, c * 128:(c + 1) * 128, :].transpose(1, 0, 2).reshape(128, 3 * D)
         for c in range(2)], axis=1)
    w2_part = w2t.reshape(2, 128, D).transpose(1, 0, 2).reshape(128, 2 * D)
    # pack3 (128, 1536+512): [f2 c0 | f2 c1 | f3]
    f2_part = np.concatenate(
        [fw2t[:, c * 128:(c + 1) * 128, :].transpose(1, 0, 2).reshape(128, 3 * D)
         for c in range(2)], axis=1)
    f3_part = fw3t.reshape(2, 128, D).transpose(1, 0, 2).reshape(128, 2 * D)
    pack3 = bf16(np.concatenate([f2_part, f3_part], axis=1))
    # pre-transposed, zero-padded conv inputs, packed with their weights
    hT_all = np.zeros((B, 128, 2, S + 2), np.float32)
    hT_all[:, :, :, 1:S + 1] = h.transpose(0, 2, 1).reshape(B, 2, 128, S).transpose(0, 2, 1, 3)
    pack2_all = bf16(np.concatenate(
        [hT_all.reshape(B, 128, 2 * (S + 2)),
         np.broadcast_to(w1_part, (B,) + w1_part.shape),
         np.broadcast_to(w2_part, (B,) + w2_part.shape)], axis=2))
    mT_all = np.zeros((B, FEAT, F + 2), np.float32)
    mT_all[:, :, 1:F + 1] = m.transpose(0, 2, 1)
    pack1_all = bf16(np.concatenate(
        [mT_all, np.broadcast_to(f1_flat, (B,) + f1_flat.shape)], axis=2))

    def chunks2(v):
        v = np.asarray(v, np.float64).reshape(2, 128)
        return v[0], v[1]

    tb1a, tb1b = chunks2(inputs["tb1"])
    tb2a, tb2b = chunks2(inputs["tb2"])
    fb1a, fb1b = chunks2(inputs["fb1"])
    fb2a, fb2b = chunks2(inputs["fb2"])
    fb3a, fb3b = chunks2(inputs["fb3"])

    k = np.arange(S)
    j = np.arange(F)
    s = np.arange(GLEN)

    in_maps = []
    for b in range(B):
        L = int(token_length[b])
        Fl = int(feat_length[b])
        C = _gln(L) + _gln(Fl + 1) - _gln(L + Fl)
        qrow = C - _gln(k + 1) - _gln(L - k)
        f2row = -_gln(j + 1) - _gln(Fl - j)
        gvec = _gln(s + 1) + _gln(L + Fl - 1 - s)
        gmat = (
            np.lib.stride_tricks.sliding_window_view(gvec, F)[:S]
            + qrow[:, None] + f2row[None, :]
        ).astype(np.float32)
        maskbias = np.where(mask[b], 0.0, NEGB)
        biaspack = np.stack(
            [tb1a, tb1b, tb2a, tb2b, fb1a, fb1b, fb2a, fb2b, fb3a, fb3b, maskbias],
            axis=1,
        ).astype(np.float32)
        in_maps.append({
            "pack1": pack1_all[b], "pack2": pack2_all[b], "pack3": pack3,
            "biaspack": biaspack, "gmat": gmat,
        })

    trace = os.environ.get("BASS_KERNEL_TRACE", "0") == "1"
    if trace:
        trace = _ensure_trace_hook()
    res = run_bass_kernel_spmd(nc, in_maps, core_ids=list(range(B)), trace=trace)
    LAST_EXEC_NS = res.exec_time_ns
    if res.instructions_and_trace is not None:
        LAST_TRACE_PATH = res.instructions_and_trace[1]

    out = np.stack([res.results[i]["out"] for i in range(B)]).astype(np.float32)
    ninf = np.float32(-np.inf)
    for b in range(B):
        L = int(token_length[b])
        Fl = int(feat_length[b])
        out[b, ~mask[b], :] = ninf
        out[b, L:, :] = ninf
        out[b, :, Fl:] = ninf
    return out


# revision 18
# speedup vs baseline: 1.1468x; 1.1468x over previous
"""Trainium2 Bass kernel for nn_AlignmentModule.

Data-parallel over batch: 8 samples -> 8 NeuronCores, one sample each.

Per-core computation (sample b):
  hh = conv1(k3, relu) -> conv2(k1) over h           (S=128, D=256)
  mm = conv1(k3, relu) -> conv2(k3, relu) -> conv3(k1) over m  (F=512, D=256)
  dist(s,f) = ||hh[s] - mm[f]||_2  via h2 + m2 - 2*hh.mm (matmul)
  log_softmax over s of (-dist) with row mask (no-max logsumexp: dist >= 0)
  out = beta_binomial_log_prior + log_softmax

Conv/dist/reduction matmuls run in bf16 (full-rate PE); the distance is
assembled in fp32 PSUM with h2 entering as the Ln activation's per-partition
bias and m2 / -lse broadcast via K=1 fp16 matmuls (small-magnitude rows).
sqrt(d2) = exp(0.5*ln(d2)) keeps every ScalarE transcendental in one
activation-table set (preloaded manually). Everything after feat-conv2 is
split into two F-halves (conv3 is 1x1 so no halo) so PE / ScalarE / VectorE
/ DMA stages of the two halves pipeline.

The beta-binomial prior factors as C + f1(k) + f2(j) + g(k+j) (all gammaln
arguments are integers); host builds a (128,512) fp32 tile
G = g(k+j) + C + f1(k) + f2(j) with math.lgamma. -inf regions
(k >= token_length, j >= feat_length) are data-independent given the length
scalars and are filled on host after the gather.
"""

import math
import os

import numpy as np

import concourse.bacc as bacc
import concourse.bass as bass
import concourse.mybir as mybir
from concourse import masks, tile
from concourse.bass_utils import run_bass_kernel_spmd

FP = mybir.dt.float32
BF = mybir.dt.bfloat16
HF = mybir.dt.float16
B, S, F, D, FEAT = 8, 128, 512, 256, 80
HH = F // 2
NEGB = -30000.0  # exp(NEGB + x) == 0 in fp32 for any realistic x
NLE_SET_ID = 6   # natural_log_exp_and_others in act_info.json

LAST_EXEC_NS = None
LAST_TRACE_PATH = None

_CACHED_NC = None


def _ensure_trace_hook():
    """Register the axon NTFF profile hook if the image's antenv lacks it."""
    import sys
    import types

    try:
        from antenv.axon_hooks import get_axon_ntff_profile_hook
        if get_axon_ntff_profile_hook() is not None:
            return True
    except ImportError:
        pass
    try:
        if "/root/.axon_site" not in sys.path:
            sys.path.insert(0, "/root/.axon_site")
        from trn_agent_boot.trn_boot import _ntff_profile_via_ctypes

        hook = _ntff_profile_via_ctypes("/opt/axon/libaxon_pjrt.so")
        mod = types.ModuleType("antenv.axon_hooks")
        holder = [hook]
        mod.get_axon_ntff_profile_hook = lambda: holder[0]
        mod.set_axon_ntff_profile_hook = lambda h: holder.__setitem__(0, h)
        sys.modules["antenv.axon_hooks"] = mod
        import concourse.bass_utils as bu

        bu.upload_artifacts = lambda tmpdir: "local://" + tmpdir
        return True
    except Exception:
        return False


def _build_nc():
    nc = bacc.Bacc("TRN2", debug=False, num_devices=B)

    def inp(name, shape, dt=FP):
        return nc.declare_dram_parameter(name, list(shape), dt, isOutput=False)

    # pack1: [mT (514) | fw1t (3*256)] on 80 partitions
    p1_d = inp("pack1", (FEAT, (F + 2) + 3 * D), BF)
    # pack2: [hT (2*130) | w1t c0 (768) | w1t c1 (768) | w2t (512)] on 128 partitions
    p2_d = inp("pack2", (128, 2 * (S + 2) + 2 * 3 * D + 2 * D), BF)
    # pack3: [fw2t c0 (768) | fw2t c1 (768) | fw3t (512)]
    p3_d = inp("pack3", (128, 2 * 3 * D + 2 * D), BF)
    bp_d = inp("biaspack", (128, 11))  # tb1x2 tb2x2 fb1x2 fb2x2 fb3x2 maskbias
    g_d = inp("gmat", (S, F))          # g(k+j) + C + f1(k) + f2(j)
    out_d = nc.declare_dram_parameter("out", [S, F], FP, isOutput=True)

    Act = mybir.ActivationFunctionType
    Alu = mybir.AluOpType

    def dap(d, offset, pairs):
        return bass.AP(d[:].tensor, offset, pairs)

    mm_ = nc.tensor.matmul

    with tile.TileContext(nc) as tc:
        with (
            tc.tile_pool(name="const", bufs=1) as cp,
            tc.tile_pool(name="wts", bufs=1) as wp,
            tc.tile_pool(name="act", bufs=1) as ap,
            tc.tile_pool(name="psA", bufs=2, space="PSUM") as psA,
            tc.tile_pool(name="psF", bufs=2, space="PSUM") as psF,
            tc.tile_pool(name="psR", bufs=1, space="PSUM") as psR,
            tc.tile_pool(name="psD", bufs=1, space="PSUM") as psD,
            tc.tile_pool(name="psB", bufs=1, space="PSUM") as psB,
        ):
            # preload the single activation-table set (ln+exp+relu+square+...)
            nc.scalar.add_instruction(
                mybir.InstLoadActFuncSet(
                    name=nc.get_next_instruction_name(),
                    act_func_set_id=NLE_SET_ID,
                    ins=[],
                    outs=[],
                )
            )

            # ---- input DMAs: three packed loads + biases + prior tile ----
            p1_sb = ap.tile([FEAT, (F + 2) + 3 * D], BF, tag="p1")
            nc.sync.dma_start(p1_sb[:], p1_d[:])
            p2_sb = ap.tile([128, 2 * (S + 2) + 2 * 3 * D + 2 * D], BF, tag="p2")
            nc.sync.dma_start(p2_sb[:], p2_d[:])
            bp_sb = cp.tile([128, 11], FP, tag="bp")
            nc.scalar.dma_start(bp_sb[:], bp_d[:])
            p3_sb = ap.tile([128, 2 * 3 * D + 2 * D], BF, tag="p3")
            nc.scalar.dma_start(p3_sb[:], p3_d[:])
            g_sb = ap.tile([S, F], FP, tag="g")
            nc.scalar.dma_start(g_sb[:], g_d[:])

            mT = p1_sb[:, 0:F + 2]
            def f1w(t, o):
                base = (F + 2) + t * D + o * 128
                return p1_sb[:, base:base + 128]
            hT = [p2_sb[:, c * (S + 2):(c + 1) * (S + 2)] for c in range(2)]
            def w1w(c, t, o):
                base = 2 * (S + 2) + c * 3 * D + t * D + o * 128
                return p2_sb[:, base:base + 128]
            def w2w(c, o):
                base = 2 * (S + 2) + 2 * 3 * D + c * D + o * 128
                return p2_sb[:, base:base + 128]
            def f2w(c, t, o):
                base = c * 3 * D + t * D + o * 128
                return p3_sb[:, base:base + 128]
            def f3w(c, o):
                base = 2 * 3 * D + c * D + o * 128
                return p3_sb[:, base:base + 128]

            tb1c = [bp_sb[:, i:i + 1] for i in (0, 1)]
            tb2c = [bp_sb[:, i:i + 1] for i in (2, 3)]
            fb1c = [bp_sb[:, i:i + 1] for i in (4, 5)]
            fb2c = [bp_sb[:, i:i + 1] for i in (6, 7)]
            fb3c = [bp_sb[:, i:i + 1] for i in (8, 9)]
            mb_col = bp_sb[:, 10:11]

            # ---- constants (warmup operands first) ----
            ones_r128 = cp.tile([1, 128], HF, tag="ones_r128")
            nc.gpsimd.memset(ones_r128[:], 1.0)
            ones_col = cp.tile([128, 1], BF, tag="ones_col")
            nc.gpsimd.memset(ones_col[:], 1.0)
            neg_r128 = cp.tile([1, 128], HF, tag="neg_r128")
            nc.gpsimd.memset(neg_r128[:], -1.0)

            pb = psB.tile([128, F], FP, tag="b")

            # ---- feat conv1 + conv2 (full width; k=3 halo) ----
            y1 = []
            for o in range(2):
                py = psF.tile([128, F], FP, tag="cf")
                for t in range(3):
                    mm_(py[:], f1w(t, o), mT[:, t:t + F],
                        start=(t == 0), stop=(t == 2))
                yp = ap.tile([128, F + 2], BF, tag=f"y1_{o}")
                nc.gpsimd.memset(yp[:, 0:1], 0.0)
                nc.gpsimd.memset(yp[:, F + 1:F + 2], 0.0)
                if o == 0:
                    nc.scalar.activation(yp[:, 1:F + 1], py[:], Act.Relu,
                                         bias=fb1c[o], scale=1.0)
                else:
                    nc.vector.tensor_scalar(
                        out=yp[:, 1:F + 1], in0=py[:], scalar1=fb1c[o], scalar2=0.0,
                        op0=Alu.add, op1=Alu.max,
                    )
                y1.append(yp)

            # ---- text conv stack (starts as soon as hT + w1 land) ----
            x1T = []
            for o in range(2):
                px = psA.tile([128, S], FP, tag="A")
                k = 0
                for t in range(3):
                    for c in range(2):
                        mm_(px[:], w1w(c, t, o),
                            hT[c][:, t:t + S], start=(k == 0), stop=(k == 5))
                        k += 1
                x = ap.tile([128, S], BF, tag=f"x1T_{o}")
                if o == 0:
                    nc.scalar.activation(x[:], px[:], Act.Relu, bias=tb1c[o], scale=1.0)
                else:
                    nc.vector.tensor_scalar(
                        out=x[:], in0=px[:], scalar1=tb1c[o], scalar2=0.0,
                        op0=Alu.add, op1=Alu.max,
                    )
                x1T.append(x)

            hhT = []
            hsq = []
            for o in range(2):
                px = psA.tile([128, S], FP, tag="A")
                for c in range(2):
                    mm_(px[:], w2w(c, o), x1T[c][:],
                        start=(c == 0), stop=(c == 1))
                hh = ap.tile([128, S], BF, tag=f"hhT_{o}")
                if o == 0:
                    nc.vector.tensor_scalar_add(hh[:], px[:], tb2c[o])
                else:
                    nc.scalar.activation(hh[:], px[:], Act.Identity, bias=tb2c[o], scale=1.0)
                hhT.append(hh)
                sq = ap.tile([128, S], BF, tag=f"hsq_{o}")
                if o == 0:
                    nc.scalar.activation(sq[:], hh[:], Act.Square)
                else:
                    nc.vector.tensor_mul(sq[:], hh[:], hh[:])
                hsq.append(sq)

            # h2 as a (128,1) column via N=1 matmuls (enters Ln as bias)
            ph2 = psA.tile([128, 1], FP, tag="A")
            for c in range(2):
                mm_(ph2[:], hsq[c][:], ones_col[:], start=(c == 0), stop=(c == 1))
            h2col = cp.tile([128, 1], FP, tag="h2col")
            nc.vector.tensor_copy(h2col[:], ph2[:])

            py2 = []
            for q in range(2):
                row = []
                for o in range(2):
                    py = psF.tile([128, HH], FP, tag="cf", name=f"c2_{q}_{o}")
                    k = 0
                    for t in range(3):
                        for c in range(2):
                            mm_(py[:], f2w(c, t, o),
                                y1[c][:, q * HH + t:q * HH + t + HH],
                                start=(k == 0), stop=(k == 5))
                            k += 1
                    row.append(py)
                py2.append(row)

            # ---- half-split pipeline: y2 relu -> conv3 -> dist -> softmax ----
            y2 = [ap.tile([128, F], BF, tag=f"y2_{o}", name=f"y2_{o}") for o in range(2)]
            mmT = [ap.tile([128, F], BF, tag=f"mmT_{o}", name=f"mmT_{o}") for o in range(2)]
            msq = [ap.tile([128, F], BF, tag=f"msq_{o}", name=f"msq_{o}") for o in range(2)]
            pm2 = psR.tile([1, F], FP, tag="rowm")
            m2n = cp.tile([1, F], HF, tag="m2n")
            pd = psD.tile([128, F], FP, tag="d")
            pcs = psR.tile([1, F], FP, tag="rowc")

            for q in range(2):
                sl = slice(q * HH, (q + 1) * HH)
                # y2 relu halves (conv3 is 1x1: no halo needed)
                for o in range(2):
                    if (o + q) % 2 == 0:
                        nc.vector.tensor_scalar(
                            out=y2[o][:, sl], in0=py2[q][o][:], scalar1=fb2c[o],
                            scalar2=0.0, op0=Alu.add, op1=Alu.max,
                        )
                    else:
                        nc.scalar.activation(y2[o][:, sl], py2[q][o][:], Act.Relu,
                                             bias=fb2c[o], scale=1.0)
                # conv3 halves
                for o in range(2):
                    pc3 = psA.tile([128, HH], FP, tag="A")
                    for c in range(2):
                        mm_(pc3[:], f3w(c, o), y2[c][:, sl],
                            start=(c == 0), stop=(c == 1))
                    if (o + q) % 2 == 0:
                        nc.scalar.activation(mmT[o][:, sl], pc3[:], Act.Identity,
                                             bias=fb3c[o], scale=1.0)
                        nc.vector.tensor_mul(msq[o][:, sl], mmT[o][:, sl], mmT[o][:, sl])
                    else:
                        nc.vector.tensor_scalar_add(mmT[o][:, sl], pc3[:], fb3c[o])
                        nc.scalar.activation(msq[o][:, sl], mmT[o][:, sl], Act.Square)

                # m2 half row, scaled by -0.5, fp16
                for c in range(2):
                    mm_(pm2[:, sl], ones_col[:], msq[c][:, sl],
                        start=(c == 0), stop=(c == 1))
                nc.vector.tensor_scalar_mul(m2n[:, sl], pm2[:, sl], -0.5)

                # dist psum = hh.mm - 0.5 m2 ; h2 enters via Ln bias
                mm_(pd[:, sl], hhT[0][:], mmT[0][:, sl], start=True, stop=False)
                mm_(pd[:, sl], hhT[1][:], mmT[1][:, sl], start=False, stop=False)
                mm_(pd[:, sl], ones_r128[:], m2n[:, sl], start=False, stop=True)

                tln = ap.tile([128, HH], FP, tag=f"tln_{q}")
                nc.scalar.activation(tln[:], pd[:, sl], Act.Ln, scale=-2.0,
                                     bias=h2col[:])
                dist = ap.tile([128, HH], FP, tag=f"dist_{q}")
                nc.scalar.activation(dist[:], tln[:], Act.Exp, scale=0.5)

                # E = exp(-dist + maskbias) (0 on masked rows)
                e_sb = ap.tile([128, HH], BF, tag=f"e_{q}")
                nc.scalar.activation(e_sb[:], dist[:], Act.Exp, bias=mb_col, scale=-1.0)

                # column logsumexp over partitions; -lse broadcast via -1 row
                mm_(pcs[:, sl], ones_col[:], e_sb[:], start=True, stop=True)
                lse = cp.tile([1, HH], HF, tag=f"lse_{q}")
                nc.scalar.activation(lse[:], pcs[:, sl], Act.Ln)
                mm_(pb[:, sl], neg_r128[:], lse[:], start=True, stop=True)

                # out = (G - dist) + pb
                fin1 = ap.tile([128, HH], FP, tag=f"fin1_{q}")
                nc.vector.scalar_tensor_tensor(
                    out=fin1[:], in0=dist[:], scalar=-1.0, in1=g_sb[:, sl],
                    op0=Alu.mult, op1=Alu.add,
                )
                out_sb = ap.tile([S, HH], FP, tag=f"outsb_{q}")
                nc.vector.tensor_add(out_sb[:], fin1[:], pb[:, sl])
                nc.sync.dma_start(out_d[:, sl], out_sb[:])

    nc.finalize()
    return nc


def _lgamma_table(n):
    # gl[i] = gammaln(i + 1) for i in 0..n-1  -> gammaln(k) = gl[k - 1]
    return np.array([math.lgamma(i + 1.0) for i in range(n)], dtype=np.float64)


_GL = _lgamma_table(2 * (S + F) + 8)
GLEN = S + F + 64  # g vector length >= S + F - 1


def _gln(x):
    # gammaln at integer x, clamped to >= 1 (gammaln(1) = 0)
    x = np.maximum(np.asarray(x, dtype=np.int64), 1)
    return _GL[x - 1]


def kernel(**inputs):
    global _CACHED_NC, LAST_EXEC_NS, LAST_TRACE_PATH
    if _CACHED_NC is None:
        _CACHED_NC = _build_nc()
    nc = _CACHED_NC

    BFNP = mybir.dt.np(BF)
    bf16 = lambda a: np.ascontiguousarray(np.asarray(a, np.float32).astype(BFNP))
    h = np.asarray(inputs["h"], np.float32)
    m = np.asarray(inputs["m"], np.float32)
    mask = np.asarray(inputs["mask"]).astype(bool)
    token_length = np.asarray(inputs["token_length"]).astype(np.int64)
    feat_length = np.asarray(inputs["feat_length"]).astype(np.int64)

    w1t = np.asarray(inputs["tw1"], np.float32).transpose(2, 1, 0)   # (3, D, D)
    w2t = np.asarray(inputs["tw2"], np.float32)[:, :, 0].T           # (D, D)
    fw1t = np.asarray(inputs["fw1"], np.float32).transpose(2, 1, 0)  # (3, 80, D)
    fw2t = np.asarray(inputs["fw2"], np.float32).transpose(2, 1, 0)  # (3, D, D)
    fw3t = np.asarray(inputs["fw3"], np.float32)[:, :, 0].T          # (D, D)
    # pack1 (80, 514+768): [mT | f1(t-major)] -- per-sample mT added below
    f1_flat = fw1t.transpose(1, 0, 2).reshape(FEAT, 3 * D)
    # pack2 (128, 260+1536+512): [hT | w1 c0 | w1 c1 | w2]
    w1_part = np.concatenate(
        [w1t[:, c * 128:(c + 1) * 128, :].transpose(1, 0, 2).reshape(128, 3 * D)
         for c in range(2)], axis=1)
    w2_part = w2t.reshape(2, 128, D).transpose(1, 0, 2).reshape(128, 2 * D)
    # pack3 (128, 1536+512): [f2 c0 | f2 c1 | f3]
    f2_part = np.concatenate(
        [fw2t[:, c * 128:(c + 1) * 128, :].transpose(1, 0, 2).reshape(128, 3 * D)
         for c in range(2)], axis=1)
    f3_part = fw3t.reshape(2, 128, D).transpose(1, 0, 2).reshape(128, 2 * D)
    pack3 = bf16(np.concatenate([f2_part, f3_part], axis=1))
    # pre-transposed, zero-padded conv inputs, packed with their weights
    hT_all = np.zeros((B, 128, 2, S + 2), np.float32)
    hT_all[:, :, :, 1:S + 1] = h.transpose(0, 2, 1).reshape(B, 2, 128, S).transpose(0, 2, 1, 3)
    pack2_all = bf16(np.concatenate(
        [hT_all.reshape(B, 128, 2 * (S + 2)),
         np.broadcast_to(w1_part, (B,) + w1_part.shape),
         np.broadcast_to(w2_part, (B,) + w2_part.shape)], axis=2))
    mT_all = np.zeros((B, FEAT, F + 2), np.float32)
    mT_all[:, :, 1:F + 1] = m.transpose(0, 2, 1)
    pack1_all = bf16(np.concatenate(
        [mT_all, np.broadcast_to(f1_flat, (B,) + f1_flat.shape)], axis=2))

    def chunks2(v):
        v = np.asarray(v, np.float64).reshape(2, 128)
        return v[0], v[1]

    tb1a, tb1b = chunks2(inputs["tb1"])
    tb2a, tb2b = chunks2(inputs["tb2"])
    fb1a, fb1b = chunks2(inputs["fb1"])
    fb2a, fb2b = chunks2(inputs["fb2"])
    fb3a, fb3b = chunks2(inputs["fb3"])

    k = np.arange(S)
    j = np.arange(F)
    s = np.arange(GLEN)

    in_maps = []
    for b in range(B):
        L = int(token_length[b])
        Fl = int(feat_length[b])
        C = _gln(L) + _gln(Fl + 1) - _gln(L + Fl)
        qrow = C - _gln(k + 1) - _gln(L - k)
        f2row = -_gln(j + 1) - _gln(Fl - j)
        gvec = _gln(s + 1) + _gln(L + Fl - 1 - s)
        gmat = (
            np.lib.stride_tricks.sliding_window_view(gvec, F)[:S]
            + qrow[:, None] + f2row[None, :]
        ).astype(np.float32)
        maskbias = np.where(mask[b], 0.0, NEGB)
        biaspack = np.stack(
            [tb1a, tb1b, tb2a, tb2b, fb1a, fb1b, fb2a, fb2b, fb3a, fb3b, maskbias],
            axis=1,
        ).astype(np.float32)
        in_maps.append({
            "pack1": pack1_all[b], "pack2": pack2_all[b], "pack3": pack3,
            "biaspack": biaspack, "gmat": gmat,
        })

    trace = os.environ.get("BASS_KERNEL_TRACE", "0") == "1"
    if trace:
        trace = _ensure_trace_hook()
    res = run_bass_kernel_spmd(nc, in_maps, core_ids=list(range(B)), trace=trace)
    LAST_EXEC_NS = res.exec_time_ns
    if res.instructions_and_trace is not None:
        LAST_TRACE_PATH = res.instructions_and_trace[1]

    out = np.stack([res.results[i]["out"] for i in range(B)]).astype(np.float32)
    ninf = np.float32(-np.inf)
    for b in range(B):
        L = int(token_length[b])
        Fl = int(feat_length[b])
        out[b, ~mask[b], :] = ninf
        out[b, L:, :] = ninf
        out[b, :, Fl:] = ninf
    return out
